# revision 8
# baseline (speedup 1.0000x reference)
"""Trainium2 Bass kernel for im2col conv2d + bias + channel-pack.

Semantics (matches the reference):
    out[c, w] = sum_k enc_x[w, k] * weight[c, k] + bias[c],  flattened to [C*W].

Strategy:
  - Shard the window dimension W=1048576 across 8 cores (131072 windows each).
  - DMA is the bottleneck (all 16 SDMA engines ~89% busy in the fp16
    baseline), so shrink bytes: input quantized to 1 B/elem on host
    (fp8e3m4 fed straight to the PE as the moving operand with fp16
    stationary weights -- verified exact on HW -- or int8 cast to fp16
    during the SWDGE DMA), output quantized to int8 with a per-channel
    scale (ACT/DVE converts round-to-nearest and saturate), dequantized
    on the host.  21.2 MB/core -> ~10.8 MB/core.
  - The output scale 1/delta_c is folded into the stationary weights and
    the bias into an extra all-ones contraction row (row 98), so psum is
    already (conv + bias)/delta_c: the psum->sbuf copies are PLAIN dtype
    converts with no operand dependencies (a [128,1] bias/scale constant
    DMA would crawl behind the bulk loads at 4 B/descriptor and stall the
    whole psum pipeline for ~15us).
  - Stationary operand is a block-diagonal [99, 128] weight matrix: rows
    0..48 = chunk-A k-values, 49..97 = chunk-B, row 98 = bias; one moving
    column covers TWO windows; two column-group matmuls (tile_position
    cols 0/64) run concurrently, each N=512 into its own half of a
    [128, 1024] fp32 psum tile ([128,1024] = 2 PSUM banks, bufs=4 covers
    all 8; copies alternate ACT / DVE so neither serializes the PE).
  - Input: ONE gpsimd SWDGE DMA per [99, f] tile, all tiles prefetched
    (bufs=n_outer).  Output: one [128, f/4] store per half o_tile on the
    sync HWDGE ring, issued as soon as its 4 copies land so store traffic
    interleaves with the load stream.  Host de-shuffles/dequantizes.
"""

import os

import numpy as np
import ml_dtypes

K = 49
C = 32
WINDOWS_NB = 1048576
N_CORES = 8
W_CORE = WINDOWS_NB // N_CORES  # 131072

F = int(os.environ.get("BASS_KERNEL_F", "16384"))  # x-columns per tile
IN_MODE = os.environ.get("BASS_IN_MODE", "fp8")     # fp8 | i8
OUT_MODE = os.environ.get("BASS_OUT_MODE", "i8")   # i8 | f16

I8_IN_CLIP = 4.0        # input int8 clip (sigmas)
I8_OUT_CLIP = 5.0       # output int8 clip (sigmas of each channel)

_PROGRAM_CACHE: dict = {}
LAST_RESULT = None  # BassKernelResults of the most recent run (for test harness)


def build_program(w_core=W_CORE, f=F, in_mode=IN_MODE, out_mode=OUT_MODE):
    import concourse.tile as tile
    from concourse import bacc, mybir

    assert w_core % (2 * f) == 0 and f % 2048 == 0
    n_outer = w_core // (2 * f)
    nq = f // 2048  # psum tiles per outer iteration
    KR = 2 * K + 1  # 98 data rows + 1 bias row

    in_dt = mybir.dt.float8e3 if in_mode == "fp8" else mybir.dt.int8
    x_sb_dt = mybir.dt.float8e3 if in_mode == "fp8" else mybir.dt.float16
    out_dt = mybir.dt.int8 if out_mode == "i8" else mybir.dt.float16

    nc = bacc.Bacc("TRN2", debug=False, num_devices=N_CORES)
    # Host-shuffled input shards (see prepare_inputs for the layout).
    xt = nc.dram_tensor("xt", [n_outer, KR, f], in_dt, kind="ExternalInput")
    w4 = nc.dram_tensor("w4", [KR, 4 * C], mybir.dt.float16, kind="ExternalInput")
    # quantized output; host dequantizes + unshuffles.
    out = nc.dram_tensor("out", [n_outer, 4 * C, f // 2], out_dt, kind="ExternalOutput")

    xbufs = min(n_outer, 4 if in_mode == "i8" else 6)
    obufs = 4
    with tile.TileContext(nc) as tc:
        with tc.tile_pool(name="const", bufs=1) as cpool, \
             tc.tile_pool(name="xin", bufs=xbufs) as xpool, \
             tc.tile_pool(name="osb", bufs=obufs) as opool, \
             tc.tile_pool(name="ps", bufs=4, space="PSUM") as ppool:
            w_sb = cpool.tile([KR, 4 * C], mybir.dt.float16)
            nc.sync.dma_start(out=w_sb, in_=w4.ap())
            # pre-warm the ACT function table so the lazy ACT_TABLE_LOAD
            # (~1.3us) runs at t~0 instead of before the first real copy
            scr = cpool.tile([1, 8], mybir.dt.float32)
            nc.gpsimd.memset(scr, 0.0)
            scr8 = cpool.tile([1, 8], out_dt)
            nc.scalar.activation(scr8, scr, mybir.ActivationFunctionType.Identity)

            xt_ap = xt.ap()
            out_ap = out.ap()

            cp = 0  # psum tile counter (for ACT/DVE alternation)
            for it in range(n_outer):
                # Bulk loads ride the gpsimd SWDGE queue.  Column-chunked:
                # (a) 4KB-row packets round-robin ~1:1 against store packets
                # on the SDMA engines (16KB packets starve the store stream
                # 4:1), (b) matmuls start after one chunk, not a whole tile.
                x_tile = xpool.tile([KR, f], x_sb_dt)
                for c4 in range(4):
                    nc.gpsimd.dma_start(
                        out=x_tile[:, c4 * (f // 4):(c4 + 1) * (f // 4)],
                        in_=xt_ap[it, :, c4 * (f // 4):(c4 + 1) * (f // 4)],
                    )
                o_tile = opool.tile([4 * C, f // 2], out_dt)
                for q in range(nq):
                    ps = ppool.tile([4 * C, 1024], mybir.dt.float32)
                    c0 = q * 2048
                    for vb in range(2):
                        pc = slice(vb * 512, (vb + 1) * 512)
                        xb = c0 + vb * 1024
                        # concurrent MM pair on PE column groups 0-1 / 2-3
                        nc.tensor.matmul(
                            ps[0:2 * C, pc], w_sb[:, 0:2 * C],
                            x_tile[:, xb:xb + 512],
                            start=True, stop=True,
                            tile_position=(0, 0),
                        )
                        nc.tensor.matmul(
                            ps[2 * C:4 * C, pc], w_sb[:, 2 * C:4 * C],
                            x_tile[:, xb + 512:xb + 1024],
                            start=True, stop=True,
                            tile_position=(0, 2 * C),
                        )
                    o_sl = o_tile[:, q * 1024:(q + 1) * 1024]
                    # plain dtype-converting copy (round-to-nearest+saturate)
                    if cp % 2 == 0:
                        nc.scalar.activation(
                            o_sl, ps, mybir.ActivationFunctionType.Identity,
                        )
                    else:
                        # immediate +0.0 add: pinned to the DVE engine
                        # (tensor_copy gets scheduled onto Scalar, which
                        # serializes all 32 copies on one engine)
                        nc.vector.tensor_scalar_add(o_sl, ps, 0.0)
                    cp += 1
                    # store each o_tile quarter as soon as its 2 copies land,
                    # so store packets interleave with the load stream
                    if q % 2 == 1:
                        c8 = (q - 1) * 1024
                        nc.sync.dma_start(
                            out=out_ap[it, :, c8:c8 + 2048],
                            in_=o_tile[:, c8:c8 + 2048],
                        )
    nc.compile()
    return nc


def _get_program():
    key = (W_CORE, F, IN_MODE, OUT_MODE)
    if key not in _PROGRAM_CACHE:
        _PROGRAM_CACHE[key] = build_program()
    return _PROGRAM_CACHE[key]


def prepare_inputs(enc_x, weight, bias, f=F, in_mode=IN_MODE, out_mode=OUT_MODE):
    """Host-side prep: per-core shuffled 1-byte shards + block-diag weights.

    Window mapping (per core): canonical window index
        w = gh*(w_core/2) + ch*(w_core/4) + it*(f/2) + q*1024 + vb*512 + t
    lands at x-tile column  X = q*2048 + vb*1024 + gh*512 + t  of iteration
    it, in x-tile row ch*49 + k (row 98 = ones for the bias), and at o_tile
    partition (2*gh+ch)*32 + c.
    """
    enc_x = np.asarray(enc_x, dtype=np.float32)
    weight = np.asarray(weight, dtype=np.float32)
    bias = np.asarray(bias, dtype=np.float32)
    n_outer = W_CORE // (2 * f)

    w_flat = weight.reshape(C, K)
    if in_mode == "fp8":
        x_enc = enc_x.astype(ml_dtypes.float8_e3m4)
        one = np.float32(1.0)
        s_in = 1.0
        enc_np_dt = ml_dtypes.float8_e3m4
    else:
        s_in = 127.0 / I8_IN_CLIP
        x_enc = np.clip(np.round(enc_x * s_in), -127, 127).astype(np.int8)
        one = np.float32(1.0)
        enc_np_dt = np.int8

    if out_mode == "i8":
        # per-channel output quantization step from a sampled conv
        ys = enc_x[:65536] @ w_flat.T + bias  # [S, C]
        delta = (I8_OUT_CLIP * ys.std(axis=0) / 127.5).astype(np.float32)  # [C]
    else:
        delta = np.ones(C, dtype=np.float32)

    # stationary matrix [99, 128]: data rows carry w/(delta_c * s_in),
    # bias row 98 carries bias_c/delta_c (the ones row is NOT pre-scaled)
    wT = (w_flat.T / (delta[None, :] * s_in)).astype(np.float16)  # [49, 32]
    brow = (bias / delta).astype(np.float16)                      # [32]
    KR = 2 * K + 1
    w4 = np.zeros((KR, 4 * C), dtype=np.float16)
    for cg in range(2):
        for ch in range(2):
            w4[ch * K:(ch + 1) * K, cg * 64 + ch * 32:cg * 64 + ch * 32 + 32] = wT
        w4[2 * K, cg * 64:cg * 64 + 32] = brow
        w4[2 * K, cg * 64 + 32:cg * 64 + 64] = brow

    shards = []
    for i in range(N_CORES):
        sh = np.ascontiguousarray(x_enc[i * W_CORE:(i + 1) * W_CORE].T)  # [49, w_core]
        # w axis -> (gh, ch, it, q, vb, t)
        arr = sh.reshape(K, 2, 2, n_outer, f // 2048, 2, 512)
        perm = arr.transpose(3, 2, 0, 4, 5, 1, 6)  # (it, ch, k, q, vb, gh, t)
        shard = np.empty((n_outer, KR, f), dtype=enc_np_dt)
        shard[:, :2 * K] = perm.reshape(n_outer, 2 * K, f)
        shard[:, 2 * K] = np.asarray(one if in_mode == "fp8" else 1, dtype=enc_np_dt)
        shards.append(shard)
    return shards, w4, delta


def kernel(enc_x, weight, bias, windows_nb=None):
    global LAST_RESULT
    from concourse import bass_utils

    shards, w4, delta = prepare_inputs(enc_x, weight, bias)
    nc = _get_program()
    in_maps = [{"xt": shards[i], "w4": w4} for i in range(N_CORES)]
    trace = bool(int(os.environ.get("BASS_KERNEL_TRACE", "0")))
    tmpdir = os.environ.get("BASS_KERNEL_TMPDIR") or None
    res = bass_utils.run_bass_kernel_spmd(
        nc, in_maps, core_ids=list(range(N_CORES)), trace=trace, tmpdir=tmpdir
    )
    LAST_RESULT = res
    n_outer = W_CORE // (2 * F)
    outs = []
    for i in range(N_CORES):
        q = res.results[i]["out"]  # [n_outer, 128, f/2]
        arr = np.asarray(q).astype(np.float32).reshape(n_outer, 2, 2, C, F // 2)
        y = arr.transpose(3, 1, 2, 0, 4).reshape(C, W_CORE)  # [c, (gh ch it u)]
        outs.append(y)
    full = np.concatenate(outs, axis=1)  # [C, W]
    full *= delta[:, None]
    return full.reshape(-1)


# revision 10
# speedup vs baseline: 1.0687x; 1.0687x over previous
"""Trainium2 Bass kernel for im2col conv2d + bias + channel-pack.

Semantics (matches the reference):
    out[c, w] = sum_k enc_x[w, k] * weight[c, k] + bias[c],  flattened to [C*W].

Strategy:
  - Shard the window dimension W=1048576 across 8 cores (131072 windows each).
  - DMA is the bottleneck (all 16 SDMA engines ~89% busy in the fp16
    baseline), so shrink bytes: input quantized to 1 B/elem on host
    (fp8e3m4 fed straight to the PE as the moving operand with fp16
    stationary weights -- verified exact on HW -- or int8 cast to fp16
    during the SWDGE DMA), output quantized to int8 with a per-channel
    scale (ACT/DVE converts round-to-nearest and saturate), dequantized
    on the host.  21.2 MB/core -> ~10.8 MB/core.
  - The output scale 1/delta_c is folded into the stationary weights and
    the bias into an extra all-ones contraction row (row 98), so psum is
    already (conv + bias)/delta_c: the psum->sbuf copies are PLAIN dtype
    converts with no operand dependencies (a [128,1] bias/scale constant
    DMA would crawl behind the bulk loads at 4 B/descriptor and stall the
    whole psum pipeline for ~15us).
  - Stationary operand is a block-diagonal [99, 128] weight matrix: rows
    0..48 = chunk-A k-values, 49..97 = chunk-B, row 98 = bias; one moving
    column covers TWO windows; two column-group matmuls (tile_position
    cols 0/64) run concurrently, each N=512 into its own half of a
    [128, 1024] fp32 psum tile ([128,1024] = 2 PSUM banks, bufs=4 covers
    all 8; copies alternate ACT / DVE so neither serializes the PE).
  - Input: ONE gpsimd SWDGE DMA per [99, f] tile, all tiles prefetched
    (bufs=n_outer).  Output: one [128, f/4] store per half o_tile on the
    sync HWDGE ring, issued as soon as its 4 copies land so store traffic
    interleaves with the load stream.  Host de-shuffles/dequantizes.
"""

import os

import numpy as np
import ml_dtypes

K = 49
C = 32
WINDOWS_NB = 1048576
N_CORES = 8
W_CORE = WINDOWS_NB // N_CORES  # 131072

F = int(os.environ.get("BASS_KERNEL_F", "16384"))  # x-columns per tile
IN_MODE = os.environ.get("BASS_IN_MODE", "fp8")     # fp8 | i8
OUT_MODE = os.environ.get("BASS_OUT_MODE", "i8")   # i8 | f16

I8_IN_CLIP = 4.0        # input int8 clip (sigmas)
I8_OUT_CLIP = 5.0       # output int8 clip (sigmas of each channel)

_PROGRAM_CACHE: dict = {}
LAST_RESULT = None  # BassKernelResults of the most recent run (for test harness)


def build_program(w_core=W_CORE, f=F, in_mode=IN_MODE, out_mode=OUT_MODE):
    import concourse.tile as tile
    from concourse import bacc, mybir

    assert w_core % (2 * f) == 0 and f % 2048 == 0
    n_outer = w_core // (2 * f)
    nq = f // 2048  # psum tiles per outer iteration
    KR = 2 * K + 1  # 98 data rows + 1 bias row

    in_dt = mybir.dt.float8e3 if in_mode == "fp8" else mybir.dt.int8
    x_sb_dt = mybir.dt.float8e3 if in_mode == "fp8" else mybir.dt.float16
    out_dt = mybir.dt.int8 if out_mode == "i8" else mybir.dt.float16

    nc = bacc.Bacc("TRN2", debug=False, num_devices=N_CORES)
    # Host-shuffled input shards (see prepare_inputs for the layout).
    xt = nc.dram_tensor("xt", [n_outer, KR, f], in_dt, kind="ExternalInput")
    w4 = nc.dram_tensor("w4", [KR, 4 * C], mybir.dt.float16, kind="ExternalInput")
    # quantized output; host dequantizes + unshuffles.
    out = nc.dram_tensor("out", [n_outer, 4 * C, f // 2], out_dt, kind="ExternalOutput")

    xbufs = min(n_outer, 4 if in_mode == "i8" else 6)
    obufs = 4
    with tile.TileContext(nc) as tc:
        with tc.tile_pool(name="const", bufs=1) as cpool, \
             tc.tile_pool(name="xin", bufs=xbufs) as xpool, \
             tc.tile_pool(name="osb", bufs=obufs) as opool, \
             tc.tile_pool(name="ps", bufs=4, space="PSUM") as ppool:
            w_sb = cpool.tile([KR, 4 * C], mybir.dt.float16)
            nc.sync.dma_start(out=w_sb, in_=w4.ap())
            # pre-warm the ACT function table so the lazy ACT_TABLE_LOAD
            # (~1.3us) runs at t~0 instead of before the first real copy
            scr = cpool.tile([1, 8], mybir.dt.float32)
            nc.gpsimd.memset(scr, 0.0)
            scr8 = cpool.tile([1, 8], out_dt)
            nc.scalar.activation(scr8, scr, mybir.ActivationFunctionType.Identity)

            xt_ap = xt.ap()
            out_ap = out.ap()

            cp = 0  # psum tile counter (for ACT/DVE alternation)
            for it in range(n_outer):
                # Bulk loads ride the gpsimd SWDGE queue.  Column-chunked:
                # (a) 4KB-row packets round-robin ~1:1 against store packets
                # on the SDMA engines (16KB packets starve the store stream
                # 4:1), (b) matmuls start after one chunk, not a whole tile.
                x_tile = xpool.tile([KR, f], x_sb_dt)
                if it == 0:
                    # extra split so the very first matmuls start early
                    cuts = [0, 2048, f // 2, f]
                else:
                    cuts = [0, f // 2, f]
                for c0_, c1_ in zip(cuts, cuts[1:]):
                    nc.gpsimd.dma_start(
                        out=x_tile[:, c0_:c1_], in_=xt_ap[it, :, c0_:c1_],
                    )
                o_tile = opool.tile([4 * C, f // 2], out_dt)
                for q in range(nq):
                    ps = ppool.tile([4 * C, 1024], mybir.dt.float32)
                    c0 = q * 2048
                    for vb in range(2):
                        pc = slice(vb * 512, (vb + 1) * 512)
                        xb = c0 + vb * 1024
                        # concurrent MM pair on PE column groups 0-1 / 2-3
                        nc.tensor.matmul(
                            ps[0:2 * C, pc], w_sb[:, 0:2 * C],
                            x_tile[:, xb:xb + 512],
                            start=True, stop=True,
                            tile_position=(0, 0),
                        )
                        nc.tensor.matmul(
                            ps[2 * C:4 * C, pc], w_sb[:, 2 * C:4 * C],
                            x_tile[:, xb + 512:xb + 1024],
                            start=True, stop=True,
                            tile_position=(0, 2 * C),
                        )
                    o_sl = o_tile[:, q * 1024:(q + 1) * 1024]
                    # plain dtype-converting copy (round-to-nearest+saturate)
                    if cp % 2 == 0:
                        nc.scalar.activation(
                            o_sl, ps, mybir.ActivationFunctionType.Identity,
                        )
                    else:
                        # immediate +0.0 add: pinned to the DVE engine
                        # (tensor_copy gets scheduled onto Scalar, which
                        # serializes all 32 copies on one engine)
                        nc.vector.tensor_scalar_add(o_sl, ps, 0.0)
                    cp += 1
                    # store each o_tile half as soon as its 4 copies land:
                    # 4KB store rows round-robin 2:1 against 8KB load rows,
                    # matching the ~61/39 load/store byte ratio
                    if q % 4 == 3:
                        c8 = (q - 3) * 1024
                        nc.sync.dma_start(
                            out=out_ap[it, :, c8:c8 + 4096],
                            in_=o_tile[:, c8:c8 + 4096],
                        )
    nc.compile()
    return nc


def _get_program():
    key = (W_CORE, F, IN_MODE, OUT_MODE)
    if key not in _PROGRAM_CACHE:
        _PROGRAM_CACHE[key] = build_program()
    return _PROGRAM_CACHE[key]


def prepare_inputs(enc_x, weight, bias, f=F, in_mode=IN_MODE, out_mode=OUT_MODE):
    """Host-side prep: per-core shuffled 1-byte shards + block-diag weights.

    Window mapping (per core): canonical window index
        w = gh*(w_core/2) + ch*(w_core/4) + it*(f/2) + q*1024 + vb*512 + t
    lands at x-tile column  X = q*2048 + vb*1024 + gh*512 + t  of iteration
    it, in x-tile row ch*49 + k (row 98 = ones for the bias), and at o_tile
    partition (2*gh+ch)*32 + c.
    """
    enc_x = np.asarray(enc_x, dtype=np.float32)
    weight = np.asarray(weight, dtype=np.float32)
    bias = np.asarray(bias, dtype=np.float32)
    n_outer = W_CORE // (2 * f)

    w_flat = weight.reshape(C, K)
    if in_mode == "fp8":
        x_enc = enc_x.astype(ml_dtypes.float8_e3m4)
        one = np.float32(1.0)
        s_in = 1.0
        enc_np_dt = ml_dtypes.float8_e3m4
    else:
        s_in = 127.0 / I8_IN_CLIP
        x_enc = np.clip(np.round(enc_x * s_in), -127, 127).astype(np.int8)
        one = np.float32(1.0)
        enc_np_dt = np.int8

    if out_mode == "i8":
        # per-channel output quantization step from a sampled conv
        ys = enc_x[:65536] @ w_flat.T + bias  # [S, C]
        delta = (I8_OUT_CLIP * ys.std(axis=0) / 127.5).astype(np.float32)  # [C]
    else:
        delta = np.ones(C, dtype=np.float32)

    # stationary matrix [99, 128]: data rows carry w/(delta_c * s_in),
    # bias row 98 carries bias_c/delta_c (the ones row is NOT pre-scaled)
    wT = (w_flat.T / (delta[None, :] * s_in)).astype(np.float16)  # [49, 32]
    brow = (bias / delta).astype(np.float16)                      # [32]
    KR = 2 * K + 1
    w4 = np.zeros((KR, 4 * C), dtype=np.float16)
    for cg in range(2):
        for ch in range(2):
            w4[ch * K:(ch + 1) * K, cg * 64 + ch * 32:cg * 64 + ch * 32 + 32] = wT
        w4[2 * K, cg * 64:cg * 64 + 32] = brow
        w4[2 * K, cg * 64 + 32:cg * 64 + 64] = brow

    shards = []
    for i in range(N_CORES):
        sh = np.ascontiguousarray(x_enc[i * W_CORE:(i + 1) * W_CORE].T)  # [49, w_core]
        # w axis -> (gh, ch, it, q, vb, t)
        arr = sh.reshape(K, 2, 2, n_outer, f // 2048, 2, 512)
        perm = arr.transpose(3, 2, 0, 4, 5, 1, 6)  # (it, ch, k, q, vb, gh, t)
        shard = np.empty((n_outer, KR, f), dtype=enc_np_dt)
        shard[:, :2 * K] = perm.reshape(n_outer, 2 * K, f)
        shard[:, 2 * K] = np.asarray(one if in_mode == "fp8" else 1, dtype=enc_np_dt)
        shards.append(shard)
    return shards, w4, delta


def kernel(enc_x, weight, bias, windows_nb=None):
    global LAST_RESULT
    from concourse import bass_utils

    shards, w4, delta = prepare_inputs(enc_x, weight, bias)
    nc = _get_program()
    in_maps = [{"xt": shards[i], "w4": w4} for i in range(N_CORES)]
    trace = bool(int(os.environ.get("BASS_KERNEL_TRACE", "0")))
    tmpdir = os.environ.get("BASS_KERNEL_TMPDIR") or None
    res = bass_utils.run_bass_kernel_spmd(
        nc, in_maps, core_ids=list(range(N_CORES)), trace=trace, tmpdir=tmpdir
    )
    LAST_RESULT = res
    n_outer = W_CORE // (2 * F)
    outs = []
    for i in range(N_CORES):
        q = res.results[i]["out"]  # [n_outer, 128, f/2]
        arr = np.asarray(q).astype(np.float32).reshape(n_outer, 2, 2, C, F // 2)
        y = arr.transpose(3, 1, 2, 0, 4).reshape(C, W_CORE)  # [c, (gh ch it u)]
        outs.append(y)
    full = np.concatenate(outs, axis=1)  # [C, W]
    full *= delta[:, None]
    return full.reshape(-1)


# revision 17
# speedup vs baseline: 1.1020x; 1.0311x over previous
"""Trainium2 Bass kernel for im2col conv2d + bias + channel-pack.

Semantics (matches the reference):
    out[c, w] = sum_k enc_x[w, k] * weight[c, k] + bias[c],  flattened to [C*W].

Strategy:
  - Shard the window dimension W=1048576 across 8 cores (131072 windows each).
  - DMA is the bottleneck (all 16 SDMA engines ~89% busy in the fp16
    baseline), so shrink bytes: input quantized to 1 B/elem on host
    (fp8e3m4 fed straight to the PE as the moving operand with fp16
    stationary weights -- verified exact on HW -- or int8 cast to fp16
    during the SWDGE DMA), output quantized to int8 with a per-channel
    scale (ACT/DVE converts round-to-nearest and saturate), dequantized
    on the host.  21.2 MB/core -> ~10.8 MB/core.
  - The output scale 1/delta_c is folded into the stationary weights and
    the bias into an extra all-ones contraction row (row 98), so psum is
    already (conv + bias)/delta_c: the psum->sbuf copies are PLAIN dtype
    converts with no operand dependencies (a [128,1] bias/scale constant
    DMA would crawl behind the bulk loads at 4 B/descriptor and stall the
    whole psum pipeline for ~15us).
  - Stationary operand is a block-diagonal [99, 128] weight matrix: rows
    0..48 = chunk-A k-values, 49..97 = chunk-B, row 98 = bias; one moving
    column covers TWO windows; two column-group matmuls (tile_position
    cols 0/64) run concurrently, each N=512 into its own half of a
    [128, 1024] fp32 psum tile ([128,1024] = 2 PSUM banks, bufs=4 covers
    all 8; copies alternate ACT / DVE so neither serializes the PE).
  - Input: ONE gpsimd SWDGE DMA per [99, f] tile, all tiles prefetched
    (bufs=n_outer).  Output: one [128, f/4] store per half o_tile on the
    sync HWDGE ring, issued as soon as its 4 copies land so store traffic
    interleaves with the load stream.  Host de-shuffles/dequantizes.
"""

import os

import numpy as np
import ml_dtypes

K = 49
C = 32
WINDOWS_NB = 1048576
N_CORES = 8
W_CORE = WINDOWS_NB // N_CORES  # 131072

F = int(os.environ.get("BASS_KERNEL_F", "16384"))  # x-columns per tile
# variable iteration schedule (units of 4096 x-columns, sums to 16):
# small head so store traffic starts flowing early, small tail so the
# final store flush is short, big middle tiles for efficient streaming
SCHED = [int(s) for s in os.environ.get("BASS_KERNEL_SCHED", "1,2,4,4,4,1").split(",")]
assert sum(SCHED) * 4096 == W_CORE // 2
IN_MODE = os.environ.get("BASS_IN_MODE", "fp8")     # fp8 | i8
OUT_MODE = os.environ.get("BASS_OUT_MODE", "i8")   # i8 | f16

I8_IN_CLIP = 4.0        # input int8 clip (sigmas)
I8_OUT_CLIP = 5.0       # output int8 clip (sigmas of each channel)

_PROGRAM_CACHE: dict = {}
LAST_RESULT = None  # BassKernelResults of the most recent run (for test harness)


def build_program(w_core=W_CORE, sched=None, in_mode=IN_MODE, out_mode=OUT_MODE):
    import concourse.tile as tile
    from concourse import bacc, mybir

    sched = sched or SCHED
    f_list = [u * 4096 for u in sched]
    n_outer = len(f_list)
    wq = w_core // 4  # windows per (gh, ch) quarter = total o-columns (32768)
    KR = 2 * K + 1  # 98 data rows + 1 bias row

    in_dt = mybir.dt.float8e3 if in_mode == "fp8" else mybir.dt.int8
    x_sb_dt = mybir.dt.float8e3 if in_mode == "fp8" else mybir.dt.float16
    out_dt = mybir.dt.int8 if out_mode == "i8" else mybir.dt.float16

    nc = bacc.Bacc("TRN2", debug=False, num_devices=N_CORES)
    # Host-shuffled input shard: iteration i occupies x-columns
    # [2*base_i, 2*base_i + f_i) (base_i in o-columns).
    xt = nc.dram_tensor("xt", [KR, 2 * wq], in_dt, kind="ExternalInput")
    w4 = nc.dram_tensor("w4", [KR, 4 * C], mybir.dt.float16, kind="ExternalInput")
    # quantized output [128, 32768]; host dequantizes + unshuffles.
    out = nc.dram_tensor("out", [4 * C, wq], out_dt, kind="ExternalOutput")

    with tile.TileContext(nc) as tc:
        with tc.tile_pool(name="const", bufs=1) as cpool, \
             tc.tile_pool(name="xin", bufs=n_outer) as xpool, \
             tc.tile_pool(name="osb", bufs=n_outer) as opool, \
             tc.tile_pool(name="ps", bufs=4, space="PSUM") as ppool:
            w_sb = cpool.tile([KR, 4 * C], mybir.dt.float16)
            nc.sync.dma_start(out=w_sb, in_=w4.ap())

            xt_ap = xt.ap()
            out_ap = out.ap()

            # issue ALL load triggers up-front (every tile has its own
            # buffer, so nothing waits on reuse); 8KB rows for the big
            # tiles so load packets stream at full rate
            x_tiles = []
            first = True
            for it, fi in enumerate(f_list):
                x_tile = xpool.tile([KR, fi], x_sb_dt)
                x_tiles.append(x_tile)
                xb0 = sum(f_list[:it])  # x-column base of this tile
                if first:
                    cuts = [0, 2048, fi]  # early start for the q=0 matmuls
                    first = False
                elif fi > 8192:
                    cuts = list(range(0, fi + 1, 8192))
                else:
                    cuts = [0, fi]
                for c0_, c1_ in zip(cuts, cuts[1:]):
                    nc.gpsimd.dma_start(
                        out=x_tile[:, c0_:c1_],
                        in_=xt_ap[:, xb0 + c0_:xb0 + c1_],
                    )

            # pre-warm the ACT function table so the lazy ACT_TABLE_LOAD
            # (~1.3us) runs during the load ramp, not before the first copy
            scr = cpool.tile([1, 8], mybir.dt.float32)
            nc.gpsimd.memset(scr, 0.0)
            scr8 = cpool.tile([1, 8], out_dt)
            nc.scalar.activation(scr8, scr, mybir.ActivationFunctionType.Identity)

            cp = 0  # psum tile counter (for ACT/DVE alternation)
            for it, fi in enumerate(f_list):
                x_tile = x_tiles[it]
                ob0 = sum(f_list[:it]) // 2  # o-column base of this iteration
                nq = fi // 2048
                o_tile = opool.tile([4 * C, fi // 2], out_dt)
                st0 = 0  # o-column of the first not-yet-stored chunk
                for q in range(nq):
                    ps = ppool.tile([4 * C, 1024], mybir.dt.float32)
                    c0 = q * 2048
                    for vb in range(2):
                        pc = slice(vb * 512, (vb + 1) * 512)
                        xb = c0 + vb * 1024
                        # concurrent MM pair on PE column groups 0-1 / 2-3
                        nc.tensor.matmul(
                            ps[0:2 * C, pc], w_sb[:, 0:2 * C],
                            x_tile[:, xb:xb + 512],
                            start=True, stop=True,
                            tile_position=(0, 0),
                        )
                        nc.tensor.matmul(
                            ps[2 * C:4 * C, pc], w_sb[:, 2 * C:4 * C],
                            x_tile[:, xb + 512:xb + 1024],
                            start=True, stop=True,
                            tile_position=(0, 2 * C),
                        )
                    o_sl = o_tile[:, q * 1024:(q + 1) * 1024]
                    # plain dtype-converting copy (round-to-nearest+saturate)
                    if cp % 2 == 0:
                        nc.scalar.activation(
                            o_sl, ps, mybir.ActivationFunctionType.Identity,
                        )
                    else:
                        # immediate +0.0 add: pinned to the DVE engine
                        # (tensor_copy gets scheduled onto Scalar, which
                        # serializes all 32 copies on one engine)
                        nc.vector.tensor_scalar_add(o_sl, ps, 0.0)
                    cp += 1
                    # store up-to-4096-col chunks as soon as their copies
                    # land, so store packets interleave with the load stream
                    oc = (q + 1) * 1024
                    if oc - st0 == 4096 or q == nq - 1:
                        nc.sync.dma_start(
                            out=out_ap[:, ob0 + st0:ob0 + oc],
                            in_=o_tile[:, st0:oc],
                        )
                        st0 = oc
    nc.compile()
    return nc


def _get_program():
    key = (W_CORE, tuple(SCHED), IN_MODE, OUT_MODE)
    if key not in _PROGRAM_CACHE:
        _PROGRAM_CACHE[key] = build_program()
    return _PROGRAM_CACHE[key]


def prepare_inputs(enc_x, weight, bias, f=F, in_mode=IN_MODE, out_mode=OUT_MODE):
    """Host-side prep: per-core shuffled 1-byte shards + block-diag weights.

    Window mapping (per core): canonical window index
        w = gh*(w_core/2) + ch*(w_core/4) + it*(f/2) + q*1024 + vb*512 + t
    lands at x-tile column  X = q*2048 + vb*1024 + gh*512 + t  of iteration
    it, in x-tile row ch*49 + k (row 98 = ones for the bias), and at o_tile
    partition (2*gh+ch)*32 + c.
    """
    enc_x = np.asarray(enc_x, dtype=np.float32)
    weight = np.asarray(weight, dtype=np.float32)
    bias = np.asarray(bias, dtype=np.float32)
    f_list = [u * 4096 for u in SCHED]
    wq = W_CORE // 4  # windows per (gh, ch) quarter

    w_flat = weight.reshape(C, K)
    if in_mode == "fp8":
        x_enc = enc_x.astype(ml_dtypes.float8_e3m4)
        one = np.float32(1.0)
        s_in = 1.0
        enc_np_dt = ml_dtypes.float8_e3m4
    else:
        s_in = 127.0 / I8_IN_CLIP
        x_enc = np.clip(np.round(enc_x * s_in), -127, 127).astype(np.int8)
        one = np.float32(1.0)
        enc_np_dt = np.int8

    if out_mode == "i8":
        # per-channel output quantization step from a sampled conv
        ys = enc_x[:65536] @ w_flat.T + bias  # [S, C]
        delta = (I8_OUT_CLIP * ys.std(axis=0) / 127.5).astype(np.float32)  # [C]
    else:
        delta = np.ones(C, dtype=np.float32)

    # stationary matrix [99, 128]: data rows carry w/(delta_c * s_in),
    # bias row 98 carries bias_c/delta_c (the ones row is NOT pre-scaled)
    wT = (w_flat.T / (delta[None, :] * s_in)).astype(np.float16)  # [49, 32]
    brow = (bias / delta).astype(np.float16)                      # [32]
    KR = 2 * K + 1
    w4 = np.zeros((KR, 4 * C), dtype=np.float16)
    for cg in range(2):
        for ch in range(2):
            w4[ch * K:(ch + 1) * K, cg * 64 + ch * 32:cg * 64 + ch * 32 + 32] = wT
        w4[2 * K, cg * 64:cg * 64 + 32] = brow
        w4[2 * K, cg * 64 + 32:cg * 64 + 64] = brow

    one_val = np.asarray(one if in_mode == "fp8" else 1, dtype=enc_np_dt)
    shards = []
    for i in range(N_CORES):
        sh = np.ascontiguousarray(x_enc[i * W_CORE:(i + 1) * W_CORE].T)  # [49, w_core]
        arr = sh.reshape(K, 2, 2, wq)  # (k, gh, ch, j)
        shard = np.empty((KR, 2 * wq), dtype=enc_np_dt)
        b = 0
        for fi in f_list:
            blk = arr[:, :, :, b // 2:(b + fi) // 2]  # [K, 2, 2, fi/2]
            blk = blk.reshape(K, 2, 2, fi // 2048, 2, 512)  # (k gh ch q vb t)
            # x-tile col = q*2048 + vb*1024 + gh*512 + t; row = ch*49 + k
            perm = blk.transpose(2, 0, 3, 4, 1, 5)  # (ch k q vb gh t)
            shard[:2 * K, b:b + fi] = perm.reshape(2 * K, fi)
            b += fi
        shard[2 * K] = one_val
        shards.append(shard)
    return shards, w4, delta


def kernel(enc_x, weight, bias, windows_nb=None):
    global LAST_RESULT
    from concourse import bass_utils

    shards, w4, delta = prepare_inputs(enc_x, weight, bias)
    nc = _get_program()
    in_maps = [{"xt": shards[i], "w4": w4} for i in range(N_CORES)]
    trace = bool(int(os.environ.get("BASS_KERNEL_TRACE", "0")))
    tmpdir = os.environ.get("BASS_KERNEL_TMPDIR") or None
    res = bass_utils.run_bass_kernel_spmd(
        nc, in_maps, core_ids=list(range(N_CORES)), trace=trace, tmpdir=tmpdir
    )
    LAST_RESULT = res
    outs = []
    for i in range(N_CORES):
        q = res.results[i]["out"]  # [128, wq]; partition = (2gh+ch)*32 + c
        arr = np.asarray(q).astype(np.float32).reshape(2, 2, C, W_CORE // 4)
        y = arr.transpose(2, 0, 1, 3).reshape(C, W_CORE)  # [c, (gh ch j)]
        outs.append(y)
    full = np.concatenate(outs, axis=1)  # [C, W]
    full *= delta[:, None]
    return full.reshape(-1)


# revision 19
# speedup vs baseline: 1.1168x; 1.0134x over previous
"""Trainium2 Bass kernel for im2col conv2d + bias + channel-pack.

Semantics (matches the reference):
    out[c, w] = sum_k enc_x[w, k] * weight[c, k] + bias[c],  flattened to [C*W].

Strategy:
  - Shard the window dimension W=1048576 across 8 cores (131072 windows each).
  - DMA is the bottleneck (all 16 SDMA engines ~89% busy in the fp16
    baseline), so shrink bytes: input quantized to 1 B/elem on host
    (fp8e3m4 fed straight to the PE as the moving operand with fp16
    stationary weights -- verified exact on HW -- or int8 cast to fp16
    during the SWDGE DMA), output quantized to int8 with a per-channel
    scale (ACT/DVE converts round-to-nearest and saturate), dequantized
    on the host.  21.2 MB/core -> ~10.8 MB/core.
  - The output scale 1/delta_c is folded into the stationary weights and
    the bias into an extra all-ones contraction row (row 98), so psum is
    already (conv + bias)/delta_c: the psum->sbuf copies are PLAIN dtype
    converts with no operand dependencies (a [128,1] bias/scale constant
    DMA would crawl behind the bulk loads at 4 B/descriptor and stall the
    whole psum pipeline for ~15us).
  - Stationary operand is a block-diagonal [99, 128] weight matrix: rows
    0..48 = chunk-A k-values, 49..97 = chunk-B, row 98 = bias; one moving
    column covers TWO windows; two column-group matmuls (tile_position
    cols 0/64) run concurrently, each N=512 into its own half of a
    [128, 1024] fp32 psum tile ([128,1024] = 2 PSUM banks, bufs=4 covers
    all 8; copies alternate ACT / DVE so neither serializes the PE).
  - Input: ONE gpsimd SWDGE DMA per [99, f] tile, all tiles prefetched
    (bufs=n_outer).  Output: one [128, f/4] store per half o_tile on the
    sync HWDGE ring, issued as soon as its 4 copies land so store traffic
    interleaves with the load stream.  Host de-shuffles/dequantizes.
"""

import os

import numpy as np
import ml_dtypes

K = 49
C = 32
WINDOWS_NB = 1048576
N_CORES = 8
W_CORE = WINDOWS_NB // N_CORES  # 131072

F = int(os.environ.get("BASS_KERNEL_F", "16384"))  # x-columns per tile
# variable iteration schedule (units of 4096 x-columns, sums to 16):
# small head so store traffic starts flowing early, small tail so the
# final store flush is short, big middle tiles for efficient streaming
SCHED = [int(s) for s in os.environ.get("BASS_KERNEL_SCHED", "4,4,4,4").split(",")]
assert sum(SCHED) * 4096 == W_CORE // 2
IN_MODE = os.environ.get("BASS_IN_MODE", "fp8")     # fp8 | i8
OUT_MODE = os.environ.get("BASS_OUT_MODE", "i8")   # i8 | f16

I8_IN_CLIP = 4.0        # input int8 clip (sigmas)
I8_OUT_CLIP = 5.0       # output int8 clip (sigmas of each channel)

_PROGRAM_CACHE: dict = {}
LAST_RESULT = None  # BassKernelResults of the most recent run (for test harness)


def build_program(w_core=W_CORE, sched=None, in_mode=IN_MODE, out_mode=OUT_MODE):
    import concourse.tile as tile
    from concourse import bacc, mybir

    sched = sched or SCHED
    f_list = [u * 4096 for u in sched]
    n_outer = len(f_list)
    wq = w_core // 4  # windows per (gh, ch) quarter = total o-columns (32768)
    KR = 2 * K + 1  # 98 data rows + 1 bias row

    in_dt = mybir.dt.float8e3 if in_mode == "fp8" else mybir.dt.int8
    x_sb_dt = mybir.dt.float8e3 if in_mode == "fp8" else mybir.dt.float16
    out_dt = mybir.dt.int8 if out_mode == "i8" else mybir.dt.float16

    nc = bacc.Bacc("TRN2", debug=False, num_devices=N_CORES)
    # Host-shuffled input shard: iteration i occupies x-columns
    # [2*base_i, 2*base_i + f_i) (base_i in o-columns).
    xt = nc.dram_tensor("xt", [KR, 2 * wq], in_dt, kind="ExternalInput")
    w4 = nc.dram_tensor("w4", [KR, 4 * C], mybir.dt.float16, kind="ExternalInput")
    # quantized output [128, 32768]; host dequantizes + unshuffles.
    out = nc.dram_tensor("out", [4 * C, wq], out_dt, kind="ExternalOutput")

    with tile.TileContext(nc) as tc:
        with tc.tile_pool(name="const", bufs=1) as cpool, \
             tc.tile_pool(name="xin", bufs=n_outer) as xpool, \
             tc.tile_pool(name="osb", bufs=n_outer) as opool, \
             tc.tile_pool(name="ps", bufs=4, space="PSUM") as ppool:
            w_sb = cpool.tile([KR, 4 * C], mybir.dt.float16)
            nc.sync.dma_start(out=w_sb, in_=w4.ap())

            xt_ap = xt.ap()
            out_ap = out.ap()

            # issue ALL load triggers up-front (every tile has its own
            # buffer, so nothing waits on reuse); 8KB rows for the big
            # tiles so load packets stream at full rate
            x_tiles = []
            first = True
            for it, fi in enumerate(f_list):
                x_tile = xpool.tile([KR, fi], x_sb_dt)
                x_tiles.append(x_tile)
                xb0 = sum(f_list[:it])  # x-column base of this tile
                if first:
                    cuts = [0, 2048, fi]  # early start for the q=0 matmuls
                    first = False
                elif fi > 8192:
                    cuts = list(range(0, fi + 1, 8192))
                else:
                    cuts = [0, fi]
                for c0_, c1_ in zip(cuts, cuts[1:]):
                    nc.gpsimd.dma_start(
                        out=x_tile[:, c0_:c1_],
                        in_=xt_ap[:, xb0 + c0_:xb0 + c1_],
                    )

            # pre-warm the ACT function table so the lazy ACT_TABLE_LOAD
            # (~1.3us) runs during the load ramp, not before the first copy
            scr = cpool.tile([1, 8], mybir.dt.float32)
            nc.gpsimd.memset(scr, 0.0)
            scr8 = cpool.tile([1, 8], out_dt)
            nc.scalar.activation(scr8, scr, mybir.ActivationFunctionType.Identity)

            cp = 0  # psum tile counter (for ACT/DVE alternation)
            for it, fi in enumerate(f_list):
                x_tile = x_tiles[it]
                ob0 = sum(f_list[:it]) // 2  # o-column base of this iteration
                nq = fi // 2048
                o_tile = opool.tile([4 * C, fi // 2], out_dt)
                st0 = 0  # o-column of the first not-yet-stored chunk
                for q in range(nq):
                    ps = ppool.tile([4 * C, 1024], mybir.dt.float32)
                    c0 = q * 2048
                    for vb in range(2):
                        pc = slice(vb * 512, (vb + 1) * 512)
                        xb = c0 + vb * 1024
                        # concurrent MM pair on PE column groups 0-1 / 2-3
                        nc.tensor.matmul(
                            ps[0:2 * C, pc], w_sb[:, 0:2 * C],
                            x_tile[:, xb:xb + 512],
                            start=True, stop=True,
                            tile_position=(0, 0),
                        )
                        nc.tensor.matmul(
                            ps[2 * C:4 * C, pc], w_sb[:, 2 * C:4 * C],
                            x_tile[:, xb + 512:xb + 1024],
                            start=True, stop=True,
                            tile_position=(0, 2 * C),
                        )
                    o_sl = o_tile[:, q * 1024:(q + 1) * 1024]
                    # plain dtype-converting copy (round-to-nearest+saturate)
                    if cp % 2 == 0:
                        nc.scalar.activation(
                            o_sl, ps, mybir.ActivationFunctionType.Identity,
                        )
                    else:
                        # immediate +0.0 add: pinned to the DVE engine
                        # (tensor_copy gets scheduled onto Scalar, which
                        # serializes all 32 copies on one engine)
                        nc.vector.tensor_scalar_add(o_sl, ps, 0.0)
                    cp += 1
                    # Stores: fat whole-o_tile DMAs (8KB rows) keep a deep
                    # store backlog on the HWDGE ring -- dense load+store
                    # mixing measures ~368 GB/s vs ~190 when stores trickle
                    # in small chunks.  The last iteration stores in 4096-col
                    # chunks so the final flush is short.
                    oc = (q + 1) * 1024
                    last_it = it == len(f_list) - 1
                    chunk = 4096 if last_it else fi // 2
                    if oc - st0 == chunk or q == nq - 1:
                        nc.sync.dma_start(
                            out=out_ap[:, ob0 + st0:ob0 + oc],
                            in_=o_tile[:, st0:oc],
                        )
                        st0 = oc
    nc.compile()
    return nc


def _get_program():
    key = (W_CORE, tuple(SCHED), IN_MODE, OUT_MODE)
    if key not in _PROGRAM_CACHE:
        _PROGRAM_CACHE[key] = build_program()
    return _PROGRAM_CACHE[key]


def prepare_inputs(enc_x, weight, bias, f=F, in_mode=IN_MODE, out_mode=OUT_MODE):
    """Host-side prep: per-core shuffled 1-byte shards + block-diag weights.

    Window mapping (per core): canonical window index
        w = gh*(w_core/2) + ch*(w_core/4) + it*(f/2) + q*1024 + vb*512 + t
    lands at x-tile column  X = q*2048 + vb*1024 + gh*512 + t  of iteration
    it, in x-tile row ch*49 + k (row 98 = ones for the bias), and at o_tile
    partition (2*gh+ch)*32 + c.
    """
    enc_x = np.asarray(enc_x, dtype=np.float32)
    weight = np.asarray(weight, dtype=np.float32)
    bias = np.asarray(bias, dtype=np.float32)
    f_list = [u * 4096 for u in SCHED]
    wq = W_CORE // 4  # windows per (gh, ch) quarter

    w_flat = weight.reshape(C, K)
    if in_mode == "fp8":
        x_enc = enc_x.astype(ml_dtypes.float8_e3m4)
        one = np.float32(1.0)
        s_in = 1.0
        enc_np_dt = ml_dtypes.float8_e3m4
    else:
        s_in = 127.0 / I8_IN_CLIP
        x_enc = np.clip(np.round(enc_x * s_in), -127, 127).astype(np.int8)
        one = np.float32(1.0)
        enc_np_dt = np.int8

    if out_mode == "i8":
        # per-channel output quantization step from a sampled conv
        ys = enc_x[:65536] @ w_flat.T + bias  # [S, C]
        delta = (I8_OUT_CLIP * ys.std(axis=0) / 127.5).astype(np.float32)  # [C]
    else:
        delta = np.ones(C, dtype=np.float32)

    # stationary matrix [99, 128]: data rows carry w/(delta_c * s_in),
    # bias row 98 carries bias_c/delta_c (the ones row is NOT pre-scaled)
    wT = (w_flat.T / (delta[None, :] * s_in)).astype(np.float16)  # [49, 32]
    brow = (bias / delta).astype(np.float16)                      # [32]
    KR = 2 * K + 1
    w4 = np.zeros((KR, 4 * C), dtype=np.float16)
    for cg in range(2):
        for ch in range(2):
            w4[ch * K:(ch + 1) * K, cg * 64 + ch * 32:cg * 64 + ch * 32 + 32] = wT
        w4[2 * K, cg * 64:cg * 64 + 32] = brow
        w4[2 * K, cg * 64 + 32:cg * 64 + 64] = brow

    one_val = np.asarray(one if in_mode == "fp8" else 1, dtype=enc_np_dt)
    shards = []
    for i in range(N_CORES):
        sh = np.ascontiguousarray(x_enc[i * W_CORE:(i + 1) * W_CORE].T)  # [49, w_core]
        arr = sh.reshape(K, 2, 2, wq)  # (k, gh, ch, j)
        shard = np.empty((KR, 2 * wq), dtype=enc_np_dt)
        b = 0
        for fi in f_list:
            blk = arr[:, :, :, b // 2:(b + fi) // 2]  # [K, 2, 2, fi/2]
            blk = blk.reshape(K, 2, 2, fi // 2048, 2, 512)  # (k gh ch q vb t)
            # x-tile col = q*2048 + vb*1024 + gh*512 + t; row = ch*49 + k
            perm = blk.transpose(2, 0, 3, 4, 1, 5)  # (ch k q vb gh t)
            shard[:2 * K, b:b + fi] = perm.reshape(2 * K, fi)
            b += fi
        shard[2 * K] = one_val
        shards.append(shard)
    return shards, w4, delta


def kernel(enc_x, weight, bias, windows_nb=None):
    global LAST_RESULT
    from concourse import bass_utils

    shards, w4, delta = prepare_inputs(enc_x, weight, bias)
    nc = _get_program()
    in_maps = [{"xt": shards[i], "w4": w4} for i in range(N_CORES)]
    trace = bool(int(os.environ.get("BASS_KERNEL_TRACE", "0")))
    tmpdir = os.environ.get("BASS_KERNEL_TMPDIR") or None
    res = bass_utils.run_bass_kernel_spmd(
        nc, in_maps, core_ids=list(range(N_CORES)), trace=trace, tmpdir=tmpdir
    )
    LAST_RESULT = res
    outs = []
    for i in range(N_CORES):
        q = res.results[i]["out"]  # [128, wq]; partition = (2gh+ch)*32 + c
        arr = np.asarray(q).astype(np.float32).reshape(2, 2, C, W_CORE // 4)
        y = arr.transpose(2, 0, 1, 3).reshape(C, W_CORE)  # [c, (gh ch j)]
        outs.append(y)
    full = np.concatenate(outs, axis=1)  # [C, W]
    full *= delta[:, None]
    return full.reshape(-1)
